# revision 34
# baseline (speedup 1.0000x reference)
"""AttentionSubsample Trainium2 kernel — data-parallel over batch on 8 cores.

v2 redesign (from baseline 282us):
  - Scores computed PRE-TRANSPOSED ([196-strip, 49] per head) by swapping
    which slice of the stacked [kT;ab | qT;I49] operand is stationary.
    exp() writes attn^T straight to SBUF as the attn@v lhsT -> the 32
    PE transposes/quad and all aT staging copies (DVE+ACT) are gone.
  - k-bias dropped entirely (constant-in-n term cancels in softmax);
    q-bias folded into the q PSUM->SBUF copy.
  - Per-head stack fill DMAs (8/quad) replaced by a 2-hop DRAM
    round-trip (2 DMAs per 8 batches) enabled by (d,h)-major feature
    ordering of the k/q projection weights.
  - k/q projections merged across batch pairs; projection matmul merged
    across the pair (lhsT [128, 98]); out copy+DMA in bf16.
  - Elementwise work spread across Pool/ACT/DVE; fine-grained emission
    interleave keeps PE fed (PE is the pacing engine).
"""

import numpy as np
import ml_dtypes

import concourse.bass as bass
import concourse.tile as tile
from concourse import bacc, mybir
from concourse.bass_utils import run_bass_kernel_spmd

BF16 = mybir.dt.bfloat16
F32 = mybir.dt.float32

B, N, NQ, C = 512, 196, 49, 256
H = 8
NCORES = 8
BPC = B // NCORES
EPS = 1e-5
SCALE = 16 ** -0.5
AF = mybir.ActivationFunctionType
ALU = mybir.AluOpType

bf16 = ml_dtypes.bfloat16


def build_core(nbatch=BPC, debug_qka=False):
    assert nbatch % 8 == 0
    npairs = nbatch // 2
    nocts = nbatch // 8
    L1, L2 = 5, 6          # B1 / B2x-mm pair lags
    nc = bacc.Bacc("TRN2", target_bir_lowering=False, debug=False)

    xt_d = nc.dram_tensor("xt", [nbatch, 2, 128, N], BF16, kind="ExternalInput")
    wkt_d = nc.dram_tensor("wkt", [2, 128, 128], BF16, kind="ExternalInput")
    wqt_d = nc.dram_tensor("wqt", [2, 128, 128], BF16, kind="ExternalInput")
    wvt_d = nc.dram_tensor("wvt", [2, 128, 256], BF16, kind="ExternalInput")
    wpt_d = nc.dram_tensor("wpt", [2, 128, 512], BF16, kind="ExternalInput")
    tq_d = nc.dram_tensor("tq", [128, 1], F32, kind="ExternalInput")
    stk_d = nc.dram_tensor("stk", [49, 8, 8, 245], BF16, kind="ExternalInput")
    id128_d = nc.dram_tensor("id128", [128, 128], BF16, kind="ExternalInput")
    tvb_d = nc.dram_tensor("tvb", [113, 256], F32, kind="ExternalInput")
    out_d = nc.dram_tensor("out", [nbatch, 49, 512], BF16, kind="ExternalOutput")

    with tile.TileContext(nc) as tc:
        with (
            tc.tile_pool(name="consts", bufs=1) as consts,
            tc.tile_pool(name="xtp", bufs=2) as xtp,
            tc.tile_pool(name="qkt", bufs=2) as qktp,
            tc.tile_pool(name="stackp", bufs=1) as stackp,
            tc.tile_pool(name="scr", bufs=2, space="DRAM") as scrp,
            tc.tile_pool(name="atp", bufs=6) as atp,
            tc.tile_pool(name="hsp", bufs=12) as hsp,
            tc.tile_pool(name="iop", bufs=3) as iop,
            tc.tile_pool(name="ps_kv", bufs=2, space="PSUM") as ps_kv,
            tc.tile_pool(name="ps_s0", bufs=1, space="PSUM") as ps_s0,
            tc.tile_pool(name="ps_s1", bufs=1, space="PSUM") as ps_s1,
            tc.tile_pool(name="ps_op", bufs=3, space="PSUM") as ps_op,
            tc.tile_pool(name="ps_t", bufs=1, space="PSUM") as ps_t,
        ):
            # ---- constants ----
            wkt_sb = consts.tile([128, 2, 128], BF16)
            wqt_sb = consts.tile([128, 2, 128], BF16)
            wvt_sb = consts.tile([128, 2, 256], BF16)
            wpt_sb = consts.tile([128, 2, 512], BF16)
            for c in range(2):
                nc.scalar.dma_start(out=wkt_sb[:, c, :], in_=wkt_d[c])
                nc.scalar.dma_start(out=wqt_sb[:, c, :], in_=wqt_d[c])
                nc.scalar.dma_start(out=wvt_sb[:, c, :], in_=wvt_d[c])
                nc.scalar.dma_start(out=wpt_sb[:, c, :], in_=wpt_d[c])
            tq_sb = consts.tile([128, 1], F32)
            nc.scalar.dma_start(out=tq_sb, in_=tq_d[:])
            id128_sb = consts.tile([128, 128], BF16)
            nc.scalar.dma_start(out=id128_sb, in_=id128_d[:])
            tvb_sb = consts.tile([113, 256], F32)
            nc.scalar.dma_start(out=tvb_sb, in_=tvb_d[:])
            zeros_sb = consts.tile([113, 256], F32)
            nc.vector.memset(zeros_sb, 0.0)

            # stacked score operands: rows 0:16 <- kT|qT (per oct via DMA2),
            # rows 16:65 <- [ab | I49] constant
            qka_tiles = []
            for i_ in range(2):
                t = stackp.tile([65, 8, 8, 245], BF16, tag=f"qka{i_}")
                nc.gpsimd.dma_start(out=t[16:65], in_=stk_d[:])
                qka_tiles.append(t)

            # v operand tiles: per pair-slot (rotation of 8 pairs)
            v_tiles = []
            for i_ in range(8):
                slot = []
                for j_ in range(2):
                    v0 = stackp.tile([128, 8, 33], BF16, tag=f"v0_{i_}{j_}")
                    v1 = stackp.tile([68, 8, 33], BF16, tag=f"v1_{i_}{j_}")
                    nc.vector.memset(v0[:, :, 32:33], 1.0)
                    nc.vector.memset(v1[:, :, 32:33], 1.0)
                    slot.append((v0, v1))
                v_tiles.append(slot)

            # aT tiles (exp outputs): per pair-slot rotation of 3
            aT_tiles = []
            for i_ in range(3):
                slot = []
                for j_ in range(2):
                    a0 = stackp.tile([128, 8, 49], BF16, tag=f"a0_{i_}{j_}")
                    a1 = stackp.tile([68, 8, 49], BF16, tag=f"a1_{i_}{j_}")
                    slot.append((a0, a1))
                aT_tiles.append(slot)

            st = {}  # per-pair state

            def xt_load(o):
                xt_sb = xtp.tile([128, 16, 196], BF16, tag=f"_xt")
                nc.sync.dma_start(
                    out=xt_sb,
                    in_=xt_d[8 * o:8 * o + 8].rearrange("b c q n -> q (b c) n"),
                )
                return xt_sb

            def a2_kq(p, xt_sb):
                """k+q projection matmuls for pair p (batches 2p, 2p+1)."""
                i = p % 4
                kv_ps = ps_kv.tile([128, 512], F32, name="kv")
                for c in range(2):
                    nc.tensor.matmul(
                        kv_ps[:, 0:392],
                        lhsT=wkt_sb[:, c, :],
                        rhs=xt_sb[:, 4 * i + c:4 * i + c + 3:2, :],
                        start=(c == 0), stop=(c == 1),
                    )
                for c in range(2):
                    xs = xt_sb[:, 4 * i + c:4 * i + c + 3:2, :].rearrange(
                        "q b (a s c2 t) -> q b a s c2 t", a=7, s=2, c2=7, t=2
                    )[:, :, :, 0, :, 0]
                    nc.tensor.matmul(
                        kv_ps[:, 392:490],
                        lhsT=wqt_sb[:, c, :],
                        rhs=xs,
                        start=(c == 0), stop=(c == 1),
                    )
                return kv_ps

            def a2_kq_copy(p, kv_ps, qkT_sb):
                i = p % 4
                # k: plain copy (k-bias cancels in softmax); q: + tq bias
                nc.scalar.activation(
                    qkT_sb[:, 2 * i:2 * i + 2, 0:196],
                    kv_ps[:, 0:392].rearrange("q (b n) -> q b n", b=2),
                    AF.Copy,
                )
                nc.scalar.activation(
                    qkT_sb[:, 2 * i:2 * i + 2, 196:245],
                    kv_ps[:, 392:490].rearrange("q (b n) -> q b n", b=2),
                    AF.Identity, bias=tq_sb, scale=1.0,
                )

            def a2_v_mm(p, b2, xt_sb, kv_ps):
                b = 2 * p + b2
                i = p % 4
                bi = 2 * (2 * i + b2)
                for c in range(2):
                    nc.tensor.matmul(
                        kv_ps[:, 0:256],
                        lhsT=xt_sb[:, bi + c, 0:128],
                        rhs=wvt_sb[:, c, :],
                        start=(c == 0), stop=(c == 1),
                    )
                for c in range(2):
                    nc.tensor.matmul(
                        kv_ps[0:68, 256:512],
                        lhsT=xt_sb[:, bi + c, 128:196],
                        rhs=wvt_sb[:, c, :],
                        start=(c == 0), stop=(c == 1),
                    )
                v0_sb, v1_sb = v_tiles[p % 8][b2]
                nc.vector.tensor_copy(
                    v0_sb[:, :, 0:32],
                    kv_ps[:, 0:256].rearrange("q (h d) -> q h d", h=8))
                nc.vector.tensor_copy(
                    v1_sb[:, :, 0:32],
                    kv_ps[0:68, 256:512].rearrange("q (h d) -> q h d", h=8))

            def b1_batch(p, b2):
                """scores + exp for batch 2p+b2 (transposed orientation)."""
                qka_sb = qka_tiles[(p // 4) % 2]
                bo = 2 * (p % 4) + b2
                a0_sb, a1_sb = aT_tiles[p % 3][b2]
                s0 = ps_s0.tile([128, 392], F32, name="s0")
                for h in range(H):
                    nc.tensor.matmul(
                        s0[:, 49 * h:49 * h + 49],
                        lhsT=qka_sb[:, h, bo, 0:128],
                        rhs=qka_sb[:, h, bo, 196:245],
                        start=True, stop=True,
                    )
                nc.scalar.activation(
                    out=a0_sb,
                    in_=s0.rearrange("n (h q) -> n h q", h=8),
                    func=AF.Exp,
                )
                s1 = ps_s1.tile([68, 392], F32, name="s1")
                for h in range(H):
                    nc.tensor.matmul(
                        s1[:, 49 * h:49 * h + 49],
                        lhsT=qka_sb[:, h, bo, 128:196],
                        rhs=qka_sb[:, h, bo, 196:245],
                        start=True, stop=True,
                    )
                nc.scalar.activation(
                    out=a1_sb,
                    in_=s1.rearrange("n (h q) -> n h q", h=8),
                    func=AF.Exp,
                )

            def b2x_mm(p):
                """attn@v matmuls for pair p."""
                op_ps = ps_op.tile([128, 512], F32, name="op")
                for b2 in range(2):
                    a0_sb, a1_sb = aT_tiles[p % 3][b2]
                    v0_sb, v1_sb = v_tiles[p % 8][b2]
                    for h in range(H):
                        nc.tensor.matmul(
                            op_ps[64 * b2:64 * b2 + 49, 33 * h:33 * h + 33],
                            lhsT=a0_sb[:, h, :],
                            rhs=v0_sb[:, h, :],
                            start=True, stop=False,
                            tile_position=(0, 64 * b2),
                        )
                        nc.tensor.matmul(
                            op_ps[64 * b2:64 * b2 + 49, 33 * h:33 * h + 33],
                            lhsT=a1_sb[:, h, :],
                            rhs=v1_sb[:, h, :],
                            start=False, stop=True,
                            tile_position=(0, 64 * b2),
                        )
                return op_ps

            def b2x_hs(p, op_ps):
                """hardswish elementwise chain for pair p (all DVE)."""
                o_view = op_ps[:, 0:264].rearrange("q (h d) -> q h d", h=8)
                zr_sb = hsp.tile([113, 8], F32, name="zr")
                nc.vector.reciprocal(zr_sb, o_view[0:113, :, 32])
                zr_b = bass.AP(tensor=zr_sb.tensor, offset=zr_sb.offset,
                               ap=[zr_sb.ap[0], zr_sb.ap[1], [0, 32]])
                y_sb = hsp.tile([113, 8, 32], F32, name="yy")
                nc.vector.tensor_mul(y_sb, o_view[0:113, :, 0:32], zr_b)
                nc.vector.tensor_add(
                    y_sb, y_sb, tvb_sb.rearrange("q (h d) -> q h d", h=8))
                r_sb = hsp.tile([113, 256], F32, name="rr")
                nc.vector.scalar_tensor_tensor(
                    out=r_sb, in0=y_sb.rearrange("q h d -> q (h d)"),
                    scalar=3.0, in1=zeros_sb,
                    op0=ALU.add, op1=ALU.max,
                )
                hs_sb = hsp.tile([113, 256], BF16, name="hs")
                nc.vector.scalar_tensor_tensor(
                    out=hs_sb, in0=r_sb, scalar=6.0,
                    in1=y_sb.rearrange("q h d -> q (h d)"),
                    op0=ALU.min, op1=ALU.mult,
                )
                return hs_sb

            def b2y_a(p, op_ps, hs_sb):
                """hs transpose + hsT staging for pair p."""
                thsT = ps_t.tile([128, 2, 128], BF16, name="tt")
                for cc in range(2):
                    nc.tensor.transpose(
                        thsT[:, cc, 0:113],
                        hs_sb[:, 128 * cc:128 * cc + 128],
                        id128_sb[0:113, 0:113])
                hsT_sb = hsp.tile([128, 2, 2, 49], BF16, name="ht")
                src = bass.AP(
                    tensor=thsT.tensor, offset=thsT.offset,
                    ap=[thsT.ap[0], [128, 2], [64, 2], [1, 49]])
                nc.vector.tensor_copy(hsT_sb, src)
                return hsT_sb

            def b2y_b(p, op_ps, hsT_sb):
                """projection for pair p."""
                for cc in range(2):
                    nc.tensor.matmul(
                        op_ps[0:98, 0:512],
                        lhsT=hsT_sb[:, cc, :, :],
                        rhs=wpt_sb[:, cc, :],
                        start=(cc == 0), stop=(cc == 1),
                    )
                return op_ps

            def b2z(p, op_ps):
                """out copy + DMA for pair p (emitted at step end so the
                proj-wait does not block the ACT queue)."""
                out_sb = iop.tile([98, 512], BF16, name="ob")
                nc.scalar.activation(out_sb, op_ps[0:98, :], AF.Copy)
                nc.sync.dma_start(
                    out=out_d[2 * p:2 * p + 2].rearrange("b q o -> (b q) o"),
                    in_=out_sb[:, :])

            def qk_redist(o, qkT_sb):
                """2-hop DRAM round-trip: head-partition redistribution."""
                scr = scrp.tile([128, 8, 245], BF16, name="scr")
                nc.sync.dma_start(out=scr, in_=qkT_sb[:, :, :])
                qka_sb = qka_tiles[o % 2]
                nc.sync.dma_start(
                    out=qka_sb[0:16],
                    in_=scr.rearrange("(d h) b n -> d h b n", d=16, h=8),
                )

            # ---------------- software pipeline ----------------
            xt_tiles = {}
            qkT_tiles = {}
            kv_tiles = {}
            b2x_state = {}
            nsteps = npairs + L2 + 3
            for s in range(nsteps):
                # prefetch xt (2 octs at start, then 3 steps ahead of use)
                if s == 0:
                    xt_tiles[0] = xt_load(0)
                    if nocts > 1:
                        xt_tiles[1] = xt_load(1)
                if s % 4 == 1 and s >= 5:
                    o_nxt = (s + 3) // 4
                    if o_nxt < nocts:
                        xt_tiles[o_nxt] = xt_load(o_nxt)

                # A2: k/q for pair s
                if s < npairs:
                    o = s // 4
                    if s % 4 == 0:
                        qkT_tiles[o] = qktp.tile([128, 8, 245], BF16,
                                                 name="qkt")
                    kv_tiles[s] = a2_kq(s, xt_tiles[o])
                    a2_kq_copy(s, kv_tiles[s], qkT_tiles[o])

                # A2: v for pair s-1, batch 0
                if 1 <= s <= npairs:
                    p = s - 1
                    a2_v_mm(p, 0, xt_tiles[p // 4], kv_tiles[p])

                # B2y for pair s-L2-2 (transposes + projection)
                if s >= L2 + 2 and s - L2 - 2 < npairs:
                    p = s - L2 - 2
                    op_ps, hs_sb = b2x_state.pop(p)
                    hsT_sb = b2y_a(p, op_ps, hs_sb)
                    b2x_state[p] = (b2y_b(p, op_ps, hsT_sb),)

                # B2x-hs for pair s-L2-1 (DVE chain, early so v-copies lead)
                if s >= L2 + 1 and s - L2 - 1 < npairs:
                    p = s - L2 - 1
                    (op_ps,) = b2x_state.pop(p)
                    b2x_state[p] = (op_ps, b2x_hs(p, op_ps))

                # B1 batch A
                if s >= L1 and s - L1 < npairs:
                    b1_batch(s - L1, 0)

                # A2: v for pair s-1, batch 1 (+ oct-end redistribution)
                if 1 <= s <= npairs:
                    p = s - 1
                    a2_v_mm(p, 1, xt_tiles[p // 4], kv_tiles.pop(p))
                    if p % 4 == 3 or p == npairs - 1:
                        o = p // 4
                        qk_redist(o, qkT_tiles.pop(o))

                # B2x-mm for pair s-L2
                if s >= L2 and s - L2 < npairs:
                    b2x_state[s - L2] = (b2x_mm(s - L2),)

                # B1 batch B
                if s >= L1 and s - L1 < npairs:
                    b1_batch(s - L1, 1)

                # B2z: out copy + DMA for pair s-L2-2 (step end)
                if s >= L2 + 2 and s - L2 - 2 < npairs:
                    p = s - L2 - 2
                    (op_ps,) = b2x_state.pop(p)
                    b2z(p, op_ps)

        if debug_qka:
            dbg_d = nc.dram_tensor("dbg", [65, 8, 8, 245], BF16,
                                   kind="ExternalOutput")
            dba_d = nc.dram_tensor("dba", [2, 2, 128, 8, 49], BF16,
                                   kind="ExternalOutput")
            with tc.tile_pool(name="dbgp", bufs=1) as dbgp:
                nc.sync.dma_start(out=dbg_d[:], in_=qka_tiles[0][:])
                for j_ in range(2):
                    a0, a1 = aT_tiles[0][j_]
                    nc.sync.dma_start(out=dba_d[0, j_], in_=a0[:])
                    nc.sync.dma_start(out=dba_d[1, j_, 0:68], in_=a1[:])

    nc.compile()
    return nc


def _build_bias_idxs():
    import itertools
    points = list(itertools.product(range(14), range(14)))
    points_ = list(itertools.product(range(7), range(7)))
    offsets, idxs = {}, []
    for p1 in points_:
        for p2 in points:
            off = (abs(p1[0] * 2 - p2[0]), abs(p1[1] * 2 - p2[1]))
            if off not in offsets:
                offsets[off] = len(offsets)
            idxs.append(offsets[off])
    return np.array(idxs, dtype=np.int32).reshape(NQ, N)


def make_inputs(x, w_kv, kv_g, kv_b, kv_m, kv_v, w_q, q_g, q_b, q_m, q_v,
                w_p, p_g, p_b, p_m, p_v, ab_table, bias_idxs, nbatch=BPC,
                ncores=NCORES):
    """Host-side preprocessing -> list of per-core input dicts."""
    f = np.float32
    x = np.asarray(x, f)
    s_kv = np.asarray(kv_g, f) / np.sqrt(np.asarray(kv_v, f) + EPS)
    wkv = np.asarray(w_kv, f) * s_kv[:, None]
    wkv_h = wkv.reshape(H, 48, C)
    w_k = wkv_h[:, :16, :]                 # [H, 16, C]
    w_v = wkv_h[:, 16:, :].reshape(256, C)
    tkv = np.asarray(kv_b, f) - np.asarray(kv_m, f) * s_kv
    t_v = tkv.reshape(H, 48)[:, 16:].reshape(256)

    s_q = np.asarray(q_g, f) / np.sqrt(np.asarray(q_v, f) + EPS)
    wq = (np.asarray(w_q, f) * (s_q * SCALE)[:, None]).reshape(H, 16, C)
    t_q = ((np.asarray(q_b, f) - np.asarray(q_m, f) * s_q) * SCALE
           ).reshape(H, 16)

    # (d, h)-major feature ordering for the k/q projections
    w_k_dh = w_k.transpose(1, 0, 2).reshape(128, C)
    w_q_dh = wq.transpose(1, 0, 2).reshape(128, C)
    t_q_dh = np.ascontiguousarray(t_q.T.reshape(128))

    s_p = np.asarray(p_g, f) / np.sqrt(np.asarray(p_v, f) + EPS)
    wp = np.asarray(w_p, f) * s_p[:, None] / 6.0
    t_p = np.asarray(p_b, f) - np.asarray(p_m, f) * s_p

    idxs = _build_bias_idxs()
    ab = np.asarray(ab_table, f)[:, idxs]                       # [8,49,196]
    ab_s = ab.transpose(1, 0, 2)                                # [49,8,196]
    qa_c = np.broadcast_to(np.eye(NQ, dtype=f)[:, None, :], (NQ, H, NQ))
    stk1 = np.concatenate([ab_s, qa_c], axis=2)                 # [49,8,245]
    stk = np.ascontiguousarray(
        np.broadcast_to(stk1[:, :, None, :], (NQ, H, 8, 245)))

    base = dict(
        wkt=np.ascontiguousarray(w_k_dh.T.reshape(2, 128, 128)).astype(bf16),
        wqt=np.ascontiguousarray(w_q_dh.T.reshape(2, 128, 128)).astype(bf16),
        wvt=np.ascontiguousarray(w_v.T.reshape(2, 128, 256)).astype(bf16),
        wpt=np.ascontiguousarray(wp.T.reshape(2, 128, 512)).astype(bf16),
        tq=np.ascontiguousarray(t_q_dh[:, None]),
        stk=stk.astype(bf16),
        id128=np.eye(128, dtype=f).astype(bf16),
        tvb=np.ascontiguousarray(np.broadcast_to(t_v, (113, 256))),
    )

    xt = x.transpose(0, 2, 1).astype(bf16).reshape(B, 2, 128, N)
    in_maps = []
    for cid in range(ncores):
        m = dict(base)
        m["xt"] = np.ascontiguousarray(xt[cid * nbatch:(cid + 1) * nbatch])
        in_maps.append(m)
    return in_maps, t_p


_NC_CACHE = {}
LAST_RESULT = None


def kernel(**inputs):
    if "nc" not in _NC_CACHE:
        _NC_CACHE["nc"] = build_core(BPC)
    nc = _NC_CACHE["nc"]
    in_maps, t_p = make_inputs(**inputs)
    res = run_bass_kernel_spmd(nc, in_maps, core_ids=list(range(NCORES)))
    global LAST_RESULT
    LAST_RESULT = res
    out = np.concatenate([r["out"] for r in res.results], axis=0)
    return out.astype(np.float32) + t_p


# revision 36
# speedup vs baseline: 1.0027x; 1.0027x over previous
"""AttentionSubsample Trainium2 kernel — data-parallel over batch on 8 cores.

v2 redesign (from baseline 282us):
  - Scores computed PRE-TRANSPOSED ([196-strip, 49] per head) by swapping
    which slice of the stacked [kT;ab | qT;I49] operand is stationary.
    exp() writes attn^T straight to SBUF as the attn@v lhsT -> the 32
    PE transposes/quad and all aT staging copies (DVE+ACT) are gone.
  - k-bias dropped entirely (constant-in-n term cancels in softmax);
    q-bias folded into the q PSUM->SBUF copy.
  - Per-head stack fill DMAs (8/quad) replaced by a 2-hop DRAM
    round-trip (2 DMAs per 8 batches) enabled by (d,h)-major feature
    ordering of the k/q projection weights.
  - k/q projections merged across batch pairs; projection matmul merged
    across the pair (lhsT [128, 98]); out copy+DMA in bf16.
  - Elementwise work spread across Pool/ACT/DVE; fine-grained emission
    interleave keeps PE fed (PE is the pacing engine).
"""

import numpy as np
import ml_dtypes

import concourse.bass as bass
import concourse.tile as tile
from concourse import bacc, mybir
from concourse.bass_utils import run_bass_kernel_spmd

BF16 = mybir.dt.bfloat16
F32 = mybir.dt.float32

B, N, NQ, C = 512, 196, 49, 256
H = 8
NCORES = 8
BPC = B // NCORES
EPS = 1e-5
SCALE = 16 ** -0.5
AF = mybir.ActivationFunctionType
ALU = mybir.AluOpType

bf16 = ml_dtypes.bfloat16


def build_core(nbatch=BPC, debug_qka=False):
    assert nbatch % 8 == 0
    npairs = nbatch // 2
    nocts = nbatch // 8
    L1, L2 = 5, 6          # B1 / B2x-mm pair lags
    nc = bacc.Bacc("TRN2", target_bir_lowering=False, debug=False)

    xt_d = nc.dram_tensor("xt", [nbatch, 2, 128, N], BF16, kind="ExternalInput")
    wkt_d = nc.dram_tensor("wkt", [2, 128, 128], BF16, kind="ExternalInput")
    wqt_d = nc.dram_tensor("wqt", [2, 128, 128], BF16, kind="ExternalInput")
    wvt_d = nc.dram_tensor("wvt", [2, 128, 256], BF16, kind="ExternalInput")
    wpt_d = nc.dram_tensor("wpt", [2, 128, 512], BF16, kind="ExternalInput")
    tq_d = nc.dram_tensor("tq", [128, 1], F32, kind="ExternalInput")
    stk_d = nc.dram_tensor("stk", [49, 8, 8, 245], BF16, kind="ExternalInput")
    id128_d = nc.dram_tensor("id128", [128, 128], BF16, kind="ExternalInput")
    tvb_d = nc.dram_tensor("tvb", [113, 256], F32, kind="ExternalInput")
    out_d = nc.dram_tensor("out", [nbatch, 49, 512], BF16, kind="ExternalOutput")

    with tile.TileContext(nc) as tc:
        with (
            tc.tile_pool(name="consts", bufs=1) as consts,
            tc.tile_pool(name="xtp", bufs=2) as xtp,
            tc.tile_pool(name="qkt", bufs=2) as qktp,
            tc.tile_pool(name="stackp", bufs=1) as stackp,
            tc.tile_pool(name="scr", bufs=2, space="DRAM") as scrp,
            tc.tile_pool(name="atp", bufs=6) as atp,
            tc.tile_pool(name="hsp", bufs=12) as hsp,
            tc.tile_pool(name="iop", bufs=3) as iop,
            tc.tile_pool(name="ps_kv", bufs=2, space="PSUM") as ps_kv,
            tc.tile_pool(name="ps_s0", bufs=1, space="PSUM") as ps_s0,
            tc.tile_pool(name="ps_s1", bufs=1, space="PSUM") as ps_s1,
            tc.tile_pool(name="ps_op", bufs=3, space="PSUM") as ps_op,
            tc.tile_pool(name="ps_t", bufs=1, space="PSUM") as ps_t,
        ):
            # ---- constants ----
            wkt_sb = consts.tile([128, 2, 128], BF16)
            wqt_sb = consts.tile([128, 2, 128], BF16)
            wvt_sb = consts.tile([128, 2, 256], BF16)
            wpt_sb = consts.tile([128, 2, 512], BF16)
            for c in range(2):
                nc.scalar.dma_start(out=wkt_sb[:, c, :], in_=wkt_d[c])
                nc.scalar.dma_start(out=wqt_sb[:, c, :], in_=wqt_d[c])
                nc.scalar.dma_start(out=wvt_sb[:, c, :], in_=wvt_d[c])
                nc.scalar.dma_start(out=wpt_sb[:, c, :], in_=wpt_d[c])
            tq_sb = consts.tile([128, 1], F32)
            nc.scalar.dma_start(out=tq_sb, in_=tq_d[:])
            id128_sb = consts.tile([128, 128], BF16)
            nc.scalar.dma_start(out=id128_sb, in_=id128_d[:])
            tvb_sb = consts.tile([113, 256], F32)
            nc.scalar.dma_start(out=tvb_sb, in_=tvb_d[:])
            zeros_sb = consts.tile([113, 256], F32)
            nc.vector.memset(zeros_sb, 0.0)

            # stacked score operands: rows 0:16 <- kT|qT (per oct via DMA2),
            # rows 16:65 <- [ab | I49] constant
            qka_tiles = []
            for i_ in range(2):
                t = stackp.tile([65, 8, 8, 245], BF16, tag=f"qka{i_}")
                nc.gpsimd.dma_start(out=t[16:65], in_=stk_d[:])
                qka_tiles.append(t)

            # v operand tiles: per pair-slot (rotation of 8 pairs)
            v_tiles = []
            for i_ in range(8):
                slot = []
                for j_ in range(2):
                    v0 = stackp.tile([128, 8, 33], BF16, tag=f"v0_{i_}{j_}")
                    v1 = stackp.tile([68, 8, 33], BF16, tag=f"v1_{i_}{j_}")
                    nc.vector.memset(v0[:, :, 32:33], 1.0)
                    nc.vector.memset(v1[:, :, 32:33], 1.0)
                    slot.append((v0, v1))
                v_tiles.append(slot)

            # aT tiles (exp outputs): per pair-slot rotation of 3
            aT_tiles = []
            for i_ in range(3):
                slot = []
                for j_ in range(2):
                    a0 = stackp.tile([128, 8, 49], BF16, tag=f"a0_{i_}{j_}")
                    a1 = stackp.tile([68, 8, 49], BF16, tag=f"a1_{i_}{j_}")
                    slot.append((a0, a1))
                aT_tiles.append(slot)

            st = {}  # per-pair state

            def xt_load(o):
                xt_sb = xtp.tile([128, 16, 196], BF16, tag=f"_xt")
                nc.sync.dma_start(
                    out=xt_sb,
                    in_=xt_d[8 * o:8 * o + 8].rearrange("b c q n -> q (b c) n"),
                )
                return xt_sb

            def a2_k_mm(p, xt_sb, kv_ps, c):
                i = p % 4
                nc.tensor.matmul(
                    kv_ps[:, 0:392],
                    lhsT=wkt_sb[:, c, :],
                    rhs=xt_sb[:, 4 * i + c:4 * i + c + 3:2, :],
                    start=(c == 0), stop=(c == 1),
                )

            def a2_q_mm(p, xt_sb, kv_ps, c):
                i = p % 4
                xs = xt_sb[:, 4 * i + c:4 * i + c + 3:2, :].rearrange(
                    "q b (a s c2 t) -> q b a s c2 t", a=7, s=2, c2=7, t=2
                )[:, :, :, 0, :, 0]
                nc.tensor.matmul(
                    kv_ps[:, 392:490],
                    lhsT=wqt_sb[:, c, :],
                    rhs=xs,
                    start=(c == 0), stop=(c == 1),
                )

            def a2_kq_copy(p, kv_ps, qkT_sb):
                i = p % 4
                # k: plain copy (k-bias cancels in softmax); q: + tq bias
                nc.scalar.activation(
                    qkT_sb[:, 2 * i:2 * i + 2, 0:196],
                    kv_ps[:, 0:392].rearrange("q (b n) -> q b n", b=2),
                    AF.Copy,
                )
                nc.scalar.activation(
                    qkT_sb[:, 2 * i:2 * i + 2, 196:245],
                    kv_ps[:, 392:490].rearrange("q (b n) -> q b n", b=2),
                    AF.Identity, bias=tq_sb, scale=1.0,
                )

            def a2_v_one(p, b2, j, xt_sb, kv_ps):
                """j in 0..3: (strip, c) v matmul for batch 2p+b2."""
                i = p % 4
                bi = 2 * (2 * i + b2)
                strip, c = divmod(j, 2)
                if strip == 0:
                    nc.tensor.matmul(
                        kv_ps[:, 0:256],
                        lhsT=xt_sb[:, bi + c, 0:128],
                        rhs=wvt_sb[:, c, :],
                        start=(c == 0), stop=(c == 1),
                    )
                else:
                    nc.tensor.matmul(
                        kv_ps[0:68, 256:512],
                        lhsT=xt_sb[:, bi + c, 128:196],
                        rhs=wvt_sb[:, c, :],
                        start=(c == 0), stop=(c == 1),
                    )

            def a2_v_copy(p, b2, kv_ps):
                v0_sb, v1_sb = v_tiles[p % 8][b2]
                nc.vector.tensor_copy(
                    v0_sb[:, :, 0:32],
                    kv_ps[:, 0:256].rearrange("q (h d) -> q h d", h=8))
                nc.vector.tensor_copy(
                    v1_sb[:, :, 0:32],
                    kv_ps[0:68, 256:512].rearrange("q (h d) -> q h d", h=8))

            def b1_batch(p, b2):
                """scores + exp for batch 2p+b2 (transposed orientation)."""
                qka_sb = qka_tiles[(p // 4) % 2]
                bo = 2 * (p % 4) + b2
                a0_sb, a1_sb = aT_tiles[p % 3][b2]
                s0 = ps_s0.tile([128, 392], F32, name="s0")
                for h in range(H):
                    nc.tensor.matmul(
                        s0[:, 49 * h:49 * h + 49],
                        lhsT=qka_sb[:, h, bo, 0:128],
                        rhs=qka_sb[:, h, bo, 196:245],
                        start=True, stop=True,
                    )
                nc.scalar.activation(
                    out=a0_sb,
                    in_=s0.rearrange("n (h q) -> n h q", h=8),
                    func=AF.Exp,
                )
                s1 = ps_s1.tile([68, 392], F32, name="s1")
                for h in range(H):
                    nc.tensor.matmul(
                        s1[:, 49 * h:49 * h + 49],
                        lhsT=qka_sb[:, h, bo, 128:196],
                        rhs=qka_sb[:, h, bo, 196:245],
                        start=True, stop=True,
                    )
                nc.scalar.activation(
                    out=a1_sb,
                    in_=s1.rearrange("n (h q) -> n h q", h=8),
                    func=AF.Exp,
                )

            def b2x_one(p, op_ps, hb):
                """attn@v strip-pair for (h, b2) index hb of pair p."""
                b2, h = divmod(hb, H)
                a0_sb, a1_sb = aT_tiles[p % 3][b2]
                v0_sb, v1_sb = v_tiles[p % 8][b2]
                nc.tensor.matmul(
                    op_ps[64 * b2:64 * b2 + 49, 33 * h:33 * h + 33],
                    lhsT=a0_sb[:, h, :],
                    rhs=v0_sb[:, h, :],
                    start=True, stop=False,
                    tile_position=(0, 64 * b2),
                    skip_group_check=True,
                )
                nc.tensor.matmul(
                    op_ps[64 * b2:64 * b2 + 49, 33 * h:33 * h + 33],
                    lhsT=a1_sb[:, h, :],
                    rhs=v1_sb[:, h, :],
                    start=False, stop=True,
                    tile_position=(0, 64 * b2),
                    skip_group_check=True,
                )

            def b2x_hs(p, op_ps):
                """hardswish elementwise chain for pair p (all DVE)."""
                o_view = op_ps[:, 0:264].rearrange("q (h d) -> q h d", h=8)
                zr_sb = hsp.tile([113, 8], F32, name="zr")
                nc.vector.reciprocal(zr_sb, o_view[0:113, :, 32])
                zr_b = bass.AP(tensor=zr_sb.tensor, offset=zr_sb.offset,
                               ap=[zr_sb.ap[0], zr_sb.ap[1], [0, 32]])
                y_sb = hsp.tile([113, 8, 32], F32, name="yy")
                nc.vector.tensor_mul(y_sb, o_view[0:113, :, 0:32], zr_b)
                nc.vector.tensor_add(
                    y_sb, y_sb, tvb_sb.rearrange("q (h d) -> q h d", h=8))
                r_sb = hsp.tile([113, 256], F32, name="rr")
                nc.vector.scalar_tensor_tensor(
                    out=r_sb, in0=y_sb.rearrange("q h d -> q (h d)"),
                    scalar=3.0, in1=zeros_sb,
                    op0=ALU.add, op1=ALU.max,
                )
                hs_sb = hsp.tile([113, 256], BF16, name="hs")
                nc.vector.scalar_tensor_tensor(
                    out=hs_sb, in0=r_sb, scalar=6.0,
                    in1=y_sb.rearrange("q h d -> q (h d)"),
                    op0=ALU.min, op1=ALU.mult,
                )
                return hs_sb

            def b2y_a(p, op_ps, hs_sb):
                """hs transpose + hsT staging for pair p."""
                thsT = ps_t.tile([128, 2, 128], BF16, name="tt")
                for cc in range(2):
                    nc.tensor.transpose(
                        thsT[:, cc, 0:113],
                        hs_sb[:, 128 * cc:128 * cc + 128],
                        id128_sb[0:113, 0:113])
                hsT_sb = hsp.tile([128, 2, 2, 49], BF16, name="ht")
                src = bass.AP(
                    tensor=thsT.tensor, offset=thsT.offset,
                    ap=[thsT.ap[0], [128, 2], [64, 2], [1, 49]])
                nc.vector.tensor_copy(hsT_sb, src)
                return hsT_sb

            def b2y_b(p, op_ps, hsT_sb):
                """projection for pair p."""
                for cc in range(2):
                    nc.tensor.matmul(
                        op_ps[0:98, 0:512],
                        lhsT=hsT_sb[:, cc, :, :],
                        rhs=wpt_sb[:, cc, :],
                        start=(cc == 0), stop=(cc == 1),
                    )
                return op_ps

            def b2z(p, op_ps):
                """out copy + DMA for pair p (emitted at step end so the
                proj-wait does not block the ACT queue)."""
                out_sb = iop.tile([98, 512], BF16, name="ob")
                nc.scalar.activation(out_sb, op_ps[0:98, :], AF.Copy)
                nc.sync.dma_start(
                    out=out_d[2 * p:2 * p + 2].rearrange("b q o -> (b q) o"),
                    in_=out_sb[:, :])

            def qk_redist(o, qkT_sb):
                """2-hop DRAM round-trip: head-partition redistribution."""
                scr = scrp.tile([128, 8, 245], BF16, name="scr")
                nc.sync.dma_start(out=scr, in_=qkT_sb[:, :, :])
                qka_sb = qka_tiles[o % 2]
                nc.sync.dma_start(
                    out=qka_sb[0:16],
                    in_=scr.rearrange("(d h) b n -> d h b n", d=16, h=8),
                )

            # ---------------- software pipeline ----------------
            # Weave: attn@v strip-pairs (short streams, 196 ld rows) are
            # interleaved 1:1 behind long-stream matmuls (k/q/v/proj) so the
            # PE weight-load of each short hides under the predecessor's
            # stream (weights are double-buffered).
            xt_tiles = {}
            qkT_tiles = {}
            kv_tiles = {}
            op_tiles = {}
            hs_state = {}
            nsteps = npairs + L2 + 3
            for s in range(nsteps):
                if s == 0:
                    xt_tiles[0] = xt_load(0)
                    if nocts > 1:
                        xt_tiles[1] = xt_load(1)
                if s % 4 == 1 and s >= 5:
                    o_nxt = (s + 3) // 4
                    if o_nxt < nocts:
                        xt_tiles[o_nxt] = xt_load(o_nxt)

                have_kq = s < npairs
                have_v = 1 <= s <= npairs
                have_b1 = L1 <= s < npairs + L1
                have_av = L2 <= s < npairs + L2
                have_hs = L2 + 1 <= s < npairs + L2 + 1
                have_b2y = L2 + 2 <= s < npairs + L2 + 2

                if have_kq and s % 4 == 0:
                    o = s // 4
                    qkT_tiles[o] = qktp.tile([128, 8, 245], BF16, name="qkt")
                if have_kq:
                    kv_tiles[s] = ps_kv.tile([128, 512], F32, name="kv")
                if have_av:
                    op_tiles[s - L2] = ps_op.tile([128, 512], F32, name="op")

                shorts = []
                if have_av:
                    pav = s - L2
                    shorts = [(pav, hb) for hb in range(16)]
                si = 0

                def pop_short(n):
                    nonlocal si
                    for _ in range(n):
                        if si < len(shorts):
                            pav, hb = shorts[si]
                            b2x_one(pav, op_tiles[pav], hb)
                            si += 1

                # --- kq weave ---
                if have_kq:
                    xt_sb = xt_tiles[s // 4]
                    a2_k_mm(s, xt_sb, kv_tiles[s], 0)
                    pop_short(1)
                    a2_k_mm(s, xt_sb, kv_tiles[s], 1)
                    pop_short(1)
                    a2_q_mm(s, xt_sb, kv_tiles[s], 0)
                    pop_short(1)
                    a2_q_mm(s, xt_sb, kv_tiles[s], 1)
                    pop_short(1)
                    a2_kq_copy(s, kv_tiles[s], qkT_tiles[s // 4])

                # --- transposes + hsT staging for pair s-L2-2 ---
                if have_b2y:
                    p = s - L2 - 2
                    op_ps = op_tiles[p]
                    hsT_sb = b2y_a(p, op_ps, hs_state.pop(p))

                # --- v batch 0 weave + copies ---
                if have_v:
                    p = s - 1
                    xt_sb = xt_tiles[p // 4]
                    for j in range(4):
                        a2_v_one(p, 0, j, xt_sb, kv_tiles[p])
                        pop_short(1)
                    a2_v_copy(p, 0, kv_tiles[p])

                # --- hardswish chain for pair s-L2-1 (DVE) ---
                if have_hs:
                    p = s - L2 - 1
                    hs_state[p] = b2x_hs(p, op_tiles[p])

                # --- B1 batch A ---
                if have_b1:
                    b1_batch(s - L1, 0)

                # --- projection weave for pair s-L2-2 ---
                if have_b2y:
                    p = s - L2 - 2
                    for cc in range(2):
                        nc.tensor.matmul(
                            op_tiles[p][0:98, 0:512],
                            lhsT=hsT_sb[:, cc, :, :],
                            rhs=wpt_sb[:, cc, :],
                            start=(cc == 0), stop=(cc == 1),
                            skip_group_check=True,
                        )
                        pop_short(1)

                # --- v batch 1 weave + copies (+ oct-end redistribution) ---
                if have_v:
                    p = s - 1
                    xt_sb = xt_tiles[p // 4]
                    for j in range(4):
                        a2_v_one(p, 1, j, xt_sb, kv_tiles[p])
                        pop_short(1)
                    a2_v_copy(p, 1, kv_tiles[p])
                    kv_tiles.pop(p)
                    if p % 4 == 3 or p == npairs - 1:
                        o = p // 4
                        qk_redist(o, qkT_tiles.pop(o))

                # --- drain remaining shorts ---
                pop_short(16)

                # --- B1 batch B ---
                if have_b1:
                    b1_batch(s - L1, 1)

                # --- out copy + DMA for pair s-L2-2 (step end) ---
                if have_b2y:
                    p = s - L2 - 2
                    b2z(p, op_tiles.pop(p))

    nc.compile()
    return nc


def _build_bias_idxs():
    import itertools
    points = list(itertools.product(range(14), range(14)))
    points_ = list(itertools.product(range(7), range(7)))
    offsets, idxs = {}, []
    for p1 in points_:
        for p2 in points:
            off = (abs(p1[0] * 2 - p2[0]), abs(p1[1] * 2 - p2[1]))
            if off not in offsets:
                offsets[off] = len(offsets)
            idxs.append(offsets[off])
    return np.array(idxs, dtype=np.int32).reshape(NQ, N)


def make_inputs(x, w_kv, kv_g, kv_b, kv_m, kv_v, w_q, q_g, q_b, q_m, q_v,
                w_p, p_g, p_b, p_m, p_v, ab_table, bias_idxs, nbatch=BPC,
                ncores=NCORES):
    """Host-side preprocessing -> list of per-core input dicts."""
    f = np.float32
    x = np.asarray(x, f)
    s_kv = np.asarray(kv_g, f) / np.sqrt(np.asarray(kv_v, f) + EPS)
    wkv = np.asarray(w_kv, f) * s_kv[:, None]
    wkv_h = wkv.reshape(H, 48, C)
    w_k = wkv_h[:, :16, :]                 # [H, 16, C]
    w_v = wkv_h[:, 16:, :].reshape(256, C)
    tkv = np.asarray(kv_b, f) - np.asarray(kv_m, f) * s_kv
    t_v = tkv.reshape(H, 48)[:, 16:].reshape(256)

    s_q = np.asarray(q_g, f) / np.sqrt(np.asarray(q_v, f) + EPS)
    wq = (np.asarray(w_q, f) * (s_q * SCALE)[:, None]).reshape(H, 16, C)
    t_q = ((np.asarray(q_b, f) - np.asarray(q_m, f) * s_q) * SCALE
           ).reshape(H, 16)

    # (d, h)-major feature ordering for the k/q projections
    w_k_dh = w_k.transpose(1, 0, 2).reshape(128, C)
    w_q_dh = wq.transpose(1, 0, 2).reshape(128, C)
    t_q_dh = np.ascontiguousarray(t_q.T.reshape(128))

    s_p = np.asarray(p_g, f) / np.sqrt(np.asarray(p_v, f) + EPS)
    wp = np.asarray(w_p, f) * s_p[:, None] / 6.0
    t_p = np.asarray(p_b, f) - np.asarray(p_m, f) * s_p

    idxs = _build_bias_idxs()
    ab = np.asarray(ab_table, f)[:, idxs]                       # [8,49,196]
    ab_s = ab.transpose(1, 0, 2)                                # [49,8,196]
    qa_c = np.broadcast_to(np.eye(NQ, dtype=f)[:, None, :], (NQ, H, NQ))
    stk1 = np.concatenate([ab_s, qa_c], axis=2)                 # [49,8,245]
    stk = np.ascontiguousarray(
        np.broadcast_to(stk1[:, :, None, :], (NQ, H, 8, 245)))

    base = dict(
        wkt=np.ascontiguousarray(w_k_dh.T.reshape(2, 128, 128)).astype(bf16),
        wqt=np.ascontiguousarray(w_q_dh.T.reshape(2, 128, 128)).astype(bf16),
        wvt=np.ascontiguousarray(w_v.T.reshape(2, 128, 256)).astype(bf16),
        wpt=np.ascontiguousarray(wp.T.reshape(2, 128, 512)).astype(bf16),
        tq=np.ascontiguousarray(t_q_dh[:, None]),
        stk=stk.astype(bf16),
        id128=np.eye(128, dtype=f).astype(bf16),
        tvb=np.ascontiguousarray(np.broadcast_to(t_v, (113, 256))),
    )

    xt = x.transpose(0, 2, 1).astype(bf16).reshape(B, 2, 128, N)
    in_maps = []
    for cid in range(ncores):
        m = dict(base)
        m["xt"] = np.ascontiguousarray(xt[cid * nbatch:(cid + 1) * nbatch])
        in_maps.append(m)
    return in_maps, t_p


_NC_CACHE = {}
LAST_RESULT = None


def kernel(**inputs):
    if "nc" not in _NC_CACHE:
        _NC_CACHE["nc"] = build_core(BPC)
    nc = _NC_CACHE["nc"]
    in_maps, t_p = make_inputs(**inputs)
    res = run_bass_kernel_spmd(nc, in_maps, core_ids=list(range(NCORES)))
    global LAST_RESULT
    LAST_RESULT = res
    out = np.concatenate([r["out"] for r in res.results], axis=0)
    return out.astype(np.float32) + t_p
